# revision 21
# baseline (speedup 1.0000x reference)
"""Multi-head self-attention (B=4, S=2048, D=1024, H=16, RoPE, causal) on 8 trn2 cores.

Sharding: core c -> batch c//2, heads [8*(c%2), 8*(c%2)+8)   (2 cores per batch,
each doing 8 of the 16 heads).  Each core computes its partial output
projection out^T [1024, 2048]; host sums the two halves per batch and
transposes back.

All matmuls run as float32r (TF32-like, full PE rate).  Layout is transposed
throughout: x^T [D,S] in SBUF, Q^T/K^T [dk,s], scores^T [k,q] (softmax sum
via a ones-column appended to V in the attn@V matmul), out^T [o,s].
"""
import sys
sys.path.insert(0, "/opt/trn_rl_repo")
import math
from contextlib import ExitStack
import numpy as np
import ml_dtypes

import concourse.bass as bass
import concourse.bacc as bacc
import concourse.mybir as mybir
from concourse.tile import TileContext
from concourse.bass_utils import run_bass_kernel_spmd

F32 = mybir.dt.float32
F32R = mybir.dt.float32r
BF16 = mybir.dt.bfloat16
ATTN_BF16 = True
ADT = BF16 if ATTN_BF16 else F32R

B, S, D, H, DK = 4, 2048, 1024, 16, 64
NCORES = 8
NPAIR = 4               # head pairs per core
QC = 512                # q chunk (matmul moving free size)
NQC = S // QC           # 4
KC = 128                # k chunk (scores psum partition dim)
NKC = S // KC           # 16
SC = 512                # s chunk for projections / outproj
NSC = S // SC           # 4

_BUILT = {}


def _build_nc():
    nc = bacc.Bacc()

    xT_d = nc.declare_dram_parameter("xT", [D, S], BF16, isOutput=False)
    wq_d = nc.declare_dram_parameter("wqT", [D, 512], BF16, isOutput=False)
    wk_d = nc.declare_dram_parameter("wkT", [D, 512], BF16, isOutput=False)
    wv_d = nc.declare_dram_parameter("wvT", [D, 512], BF16, isOutput=False)
    wo_d = nc.declare_dram_parameter("woT", [512, D], BF16, isOutput=False)
    ctab_d = nc.declare_dram_parameter("ctab", [128, S], F32, isOutput=False)
    stab_d = nc.declare_dram_parameter("stab", [128, S], F32, isOutput=False)
    mk_d = nc.declare_dram_parameter("masks", [KC, 4, QC], ADT, isOutput=False)
    idT_d = nc.declare_dram_parameter("identT", [128, 64], F32, isOutput=False)
    ones16_d = nc.declare_dram_parameter("ones16", [128, NKC], ADT, isOutput=False)
    out_d = nc.declare_dram_parameter("outP", [D, S], F32, isOutput=True)

    swapmask = [i ^ 1 for i in range(32)]

    with TileContext(nc) as tc, ExitStack() as ctx:
        ep = ctx.enter_context
        consts = ep(tc.tile_pool(name="consts", bufs=1))
        xpool = ep(tc.tile_pool(name="xt", bufs=8))
        wpool = ep(tc.tile_pool(name="w", bufs=96))
        wopool = ep(tc.tile_pool(name="wo", bufs=32))
        vpool = ep(tc.tile_pool(name="vraw", bufs=1))
        rupool = ep(tc.tile_pool(name="ru", bufs=2))
        rvpool = ep(tc.tile_pool(name="rv", bufs=2))
        qkpool = ep(tc.tile_pool(name="qk", bufs=4))
        v1pool = ep(tc.tile_pool(name="v1", bufs=3))
        ppool = ep(tc.tile_pool(name="pT", bufs=2))
        orawpool = ep(tc.tile_pool(name="oraw", bufs=1))
        sumpool = ep(tc.tile_pool(name="sums", bufs=2))
        recpool = ep(tc.tile_pool(name="rec", bufs=2))
        otpool = ep(tc.tile_pool(name="oT", bufs=4))
        obpool = ep(tc.tile_pool(name="ob", bufs=4))
        drpool = ep(tc.tile_pool(name="dr", bufs=4, space="DRAM"))
        psA = ep(tc.tile_pool(name="psA", bufs=2, space="PSUM"))
        psB = ep(tc.tile_pool(name="psB", bufs=2, space="PSUM"))
        psO = ep(tc.tile_pool(name="psO", bufs=2, space="PSUM"))

        # resident x^T: 8 tiles [128, S] bf16 (4KB/partition each) — first,
        # so the first projection matmuls can start ASAP
        xres = []
        for ic in range(8):
            xt = xpool.tile([128, S], BF16, tag="xt")
            nc.sync.dma_start(out=xt, in_=xT_d[ic * 128:(ic + 1) * 128, :])
            xres.append(xt)

        # prefetch ALL projection + output weights up front (bf16, small)
        wts_all = []
        for hp in range(NPAIR):
            wts = {}
            for pj, wsrc in enumerate((wq_d, wk_d, wv_d)):
                for ic in range(8):
                    w = wpool.tile([128, 128], BF16, tag="w")
                    nc.sync.dma_start(
                        out=w, in_=wsrc[ic * 128:(ic + 1) * 128,
                                        hp * 128:(hp + 1) * 128])
                    wts[(pj, ic)] = w
            wts_all.append(wts)
        wos_all = []
        for oc in range(8):
            wos = []
            for hp in range(NPAIR):
                w = wopool.tile([128, 128], BF16, tag="wo")
                nc.sync.dma_start(
                    out=w, in_=wo_d[hp * 128:(hp + 1) * 128,
                                    oc * 128:(oc + 1) * 128])
                wos.append(w)
            wos_all.append(wos)

        # rope tables, split per s-chunk so rope of chunk 0 starts early
        ctabs, stabs = [], []
        for sc in range(NSC):
            ct = consts.tile([128, SC], F32, tag=f"ct{sc}")
            st = consts.tile([128, SC], F32, tag=f"st{sc}")
            nc.sync.dma_start(out=ct, in_=ctab_d[:, sc * SC:(sc + 1) * SC])
            nc.sync.dma_start(out=st, in_=stab_d[:, sc * SC:(sc + 1) * SC])
            ctabs.append(ct)
            stabs.append(st)
        masks = consts.tile([KC, 4, QC], ADT)
        identT = consts.tile([128, 64], F32)
        nc.sync.dma_start(out=masks, in_=mk_d[:, :, :])
        nc.sync.dma_start(out=identT, in_=idT_d[:, :])

        oTs = []
        for hp in range(NPAIR):
            # ---------------- projections for head pair hp ----------------
            wts = wts_all[hp]

            qT = qkpool.tile([128, S], ADT, tag="qk")
            kT = qkpool.tile([128, S], ADT, tag="qk")
            vraw = vpool.tile([128, S], F32, tag="vraw")

            _sc = nc.named_scope(f"proj{hp}"); _sc.__enter__()
            for sc in range(NSC):
                ssl = slice(sc * SC, (sc + 1) * SC)
                for pj in range(3):
                    ps = psA.tile([128, SC], F32, tag="psA")
                    for ic in range(8):
                        nc.tensor.matmul(ps, wts[(pj, ic)], xres[ic][:, ssl],
                                         start=(ic == 0), stop=(ic == 7))
                    if pj < 2:  # Q or K: RoPE directly from PSUM
                        dst = qT if pj == 0 else kT
                        sh = rupool.tile([128, SC], F32, tag="ru")
                        nc.vector.stream_shuffle(out=sh, in_=ps, mask=swapmask)
                        t1 = rvpool.tile([128, SC], F32, tag="rv")
                        nc.vector.tensor_mul(out=t1, in0=ps, in1=ctabs[sc])
                        t2 = rupool.tile([128, SC], F32, tag="ru")
                        nc.vector.tensor_mul(out=t2, in0=sh, in1=stabs[sc])
                        nc.vector.tensor_add(out=dst[:, ssl], in0=t1, in1=t2)
                    else:       # V: drain to SBUF for PE transpose
                        nc.scalar.copy(out=vraw[:, ssl], in_=ps)

            _sc.__exit__(None, None, None)
            # ---------------- V transpose: [dk, s] -> [s, dk] + ones col ----
            _sc = nc.named_scope(f"vt{hp}"); _sc.__enter__()
            v1s = []
            for h in range(2):
                v1 = v1pool.tile([128, NKC, 65], ADT, tag="v1")
                for half in range(2):
                    pvt = psB.tile([128, 512], F32, tag="big")
                    for j in range(8):
                        kc = half * 8 + j
                        nc.tensor.transpose(
                            pvt[:, j * 64:(j + 1) * 64],
                            vraw[h * 64:(h + 1) * 64, kc * 128:(kc + 1) * 128],
                            identT[h * 64:(h + 1) * 64, 0:64])
                    nc.vector.tensor_copy(
                        out=v1[:, half * 8:(half + 1) * 8, 0:64],
                        in_=pvt.rearrange("p (kc d) -> p kc d", d=64))
                nc.sync.dma_start(out=v1[:, :, 64:65],
                                  in_=ones16_d[:, :].unsqueeze(2))
                v1s.append(v1)

            _sc.__exit__(None, None, None)
            # ---------------- attention per head ----------------
            _sc = nc.named_scope(f"attn{hp}"); _sc.__enter__()
            oT = otpool.tile([128, S], ADT, tag="oT")
            oraw = orawpool.tile([128, S], F32, tag="oraw")
            for h in range(2):
                hs = slice(h * 64, (h + 1) * 64)
                sums = sumpool.tile([128, QC], F32, tag="sums")
                nc.vector.memset(sums, 1.0)
                for qc in range(NQC):
                    nact = 4 * qc + 4
                    qsl = slice(qc * QC, (qc + 1) * QC)
                    pquads = []
                    for pr in range(nact // 2):
                        psq = psB.tile([128, 1024], F32, tag="big")
                        for sl in range(2):
                            kc = pr * 2 + sl
                            csl = slice(sl * QC, (sl + 1) * QC)
                            nc.tensor.matmul(
                                psq[:, csl],
                                kT[hs, kc * KC:(kc + 1) * KC],
                                qT[hs, qsl],
                                start=True, stop=True)
                        pq = ppool.tile([128, 1024], ADT, tag="pT")
                        nc.scalar.activation(
                            out=pq, in_=psq,
                            func=mybir.ActivationFunctionType.Exp, scale=0.125)
                        for sl in range(2):
                            kc = pr * 2 + sl
                            moff = kc - 4 * qc
                            if moff >= 0:   # causal boundary crosses: zero masked
                                csl = slice(sl * QC, (sl + 1) * QC)
                                nc.vector.tensor_mul(
                                    out=pq[:, csl], in0=pq[:, csl],
                                    in1=masks[:, moff, :])
                        pquads.append(pq)
                    pso = psO.tile([65, QC], F32, tag="psO")
                    for kc in range(nact):
                        pr, sl = divmod(kc, 2)
                        nc.tensor.matmul(
                            pso, v1s[h][:, kc, :],
                            pquads[pr][:, sl * QC:(sl + 1) * QC],
                            start=(kc == 0), stop=(kc == nact - 1))
                    nc.vector.tensor_copy(out=oraw[hs, qsl], in_=pso[0:64, :])
                    nc.vector.tensor_copy(out=sums[32 * qc:32 * qc + 1, :],
                                          in_=pso[64:65, :])
                # batched normalization for this head
                rec = recpool.tile([128, QC], F32, tag="rec")
                nc.vector.reciprocal(out=rec, in_=sums)
                drt = drpool.tile([NQC, QC], F32)
                for qc in range(NQC):
                    nc.sync.dma_start(out=drt[qc:qc + 1, :],
                                      in_=rec[32 * qc:32 * qc + 1, :])
                for qc in range(NQC):
                    qsl = slice(qc * QC, (qc + 1) * QC)
                    recB = recpool.tile([128, QC], F32, tag="recB")
                    nc.sync.dma_start(out=recB[hs, :],
                                      in_=drt[qc:qc + 1, :].to_broadcast((64, QC)))
                    nc.vector.tensor_mul(out=oT[hs, qsl], in0=oraw[hs, qsl],
                                         in1=recB[hs, :])
            _sc.__exit__(None, None, None)
            oTs.append(oT)

        # ---------------- output projection ----------------
        _sc = nc.named_scope("outproj"); _sc.__enter__()
        for oc in range(8):
            wos = wos_all[oc]
            for sc in range(NSC):
                ps = psA.tile([128, SC], F32, tag="psA")
                for hp in range(NPAIR):
                    nc.tensor.matmul(ps, wos[hp],
                                     oTs[hp][:, sc * SC:(sc + 1) * SC],
                                     start=(hp == 0), stop=(hp == NPAIR - 1))
                ob = obpool.tile([128, SC], F32, tag="ob")
                if sc % 2 == 0:
                    nc.vector.tensor_copy(out=ob, in_=ps)
                else:
                    nc.scalar.copy(out=ob, in_=ps)
                nc.sync.dma_start(
                    out=out_d[oc * 128:(oc + 1) * 128, sc * SC:(sc + 1) * SC],
                    in_=ob)
        _sc.__exit__(None, None, None)

    nc.compile()
    return nc


def get_nc():
    if "nc" not in _BUILT:
        _BUILT["nc"] = _build_nc()
    return _BUILT["nc"]


def _host_prep(x, Wq, Wk, Wv, Wo, token_positions):
    pos = np.asarray(token_positions).astype(np.float32)
    half = DK // 2
    inv_freq = 1.0 / (10000.0 ** (np.arange(half, dtype=np.float32) * 2.0 / DK))
    ang = pos[:, None] * inv_freq[None, :]          # [S, 32]
    cos = np.cos(ang).astype(np.float32)            # [S, 32]
    sin = np.sin(ang).astype(np.float32)
    p = np.arange(128)
    j = (p % 64) // 2
    sign = np.where(p % 2 == 0, -1.0, 1.0).astype(np.float32)
    ctab = np.ascontiguousarray(cos[:, j].T)                      # [128, S]
    stab = np.ascontiguousarray(sin[:, j].T * sign[:, None])      # [128, S]

    kk = np.arange(KC)[:, None]
    qq = np.arange(QC)[None, :]
    adt = ml_dtypes.bfloat16 if ATTN_BF16 else np.float32
    masks = np.stack([np.where(qq >= kk + 128 * m, 1.0, 0.0)
                      for m in range(4)], axis=1).astype(adt)  # [KC,4,QC]
    identT = np.vstack([np.eye(64, dtype=np.float32)] * 2)
    ones16 = np.ones((128, NKC), dtype=adt)

    bf = ml_dtypes.bfloat16
    in_maps = []
    for c in range(NCORES):
        b, hf = divmod(c, 2)
        m = {}
        m["xT"] = np.ascontiguousarray(x[b].T).astype(bf)  # [D, S]
        m["wqT"] = np.ascontiguousarray(Wq[hf * 512:(hf + 1) * 512, :].T).astype(bf)
        m["wkT"] = np.ascontiguousarray(Wk[hf * 512:(hf + 1) * 512, :].T).astype(bf)
        m["wvT"] = np.ascontiguousarray(Wv[hf * 512:(hf + 1) * 512, :].T).astype(bf)
        m["woT"] = np.ascontiguousarray(Wo[:, hf * 512:(hf + 1) * 512].T).astype(bf)
        m["ctab"] = ctab
        m["stab"] = stab
        m["masks"] = masks
        m["identT"] = identT
        m["ones16"] = ones16
        in_maps.append(m)
    return in_maps


def run(inputs, trace=False, **kw):
    in_maps = _host_prep(**{k: np.asarray(v) for k, v in inputs.items()})
    nc = get_nc()
    res = run_bass_kernel_spmd(nc, in_maps, list(range(NCORES)), trace=trace, **kw)
    outs = [res.results[c]["outP"] for c in range(NCORES)]
    out = np.stack([(outs[2 * b] + outs[2 * b + 1]).T for b in range(B)])
    return out.astype(np.float32), res


def kernel(**inputs):
    out, _ = run(inputs, trace=False)
    return out



# revision 25
# speedup vs baseline: 1.0298x; 1.0298x over previous
"""Multi-head self-attention (B=4, S=2048, D=1024, H=16, RoPE, causal) on 8 trn2 cores.

Sharding: core c -> batch c//2, heads [8*(c%2), 8*(c%2)+8)   (2 cores per batch,
each doing 8 of the 16 heads).  Each core computes its partial output
projection out^T [1024, 2048]; host sums the two halves per batch and
transposes back.

All matmuls bf16 (f32 psum).  Layout is transposed throughout: x^T [D,S] in
SBUF (resident), Q^T/K^T [dk,s], scores^T [k,q] (softmax sum via a
ones-column appended to V in the attn@V matmul), out^T [o,s].

Scores matmuls have K=64 contraction: the two heads of a pair run
concurrently in PE row groups (0,0)/(64,0) via base-partition-derived
tile_position.  Causal masking: 0/1 bf16 multiply on P (gpsimd) instead of
-1e9 matmul adds.
"""
import sys
sys.path.insert(0, "/opt/trn_rl_repo")
import math
from contextlib import ExitStack
import numpy as np
import ml_dtypes

import concourse.bass as bass
import concourse.bacc as bacc
import concourse.mybir as mybir
from concourse.tile import TileContext
from concourse.bass_utils import run_bass_kernel_spmd

F32 = mybir.dt.float32
BF16 = mybir.dt.bfloat16
ADT = BF16

B, S, D, H, DK = 4, 2048, 1024, 16, 64
NCORES = 8
NPAIR = 4               # head pairs per core
QC = 512                # q chunk (matmul moving free size)
NQC = S // QC           # 4
KC = 128                # k chunk (scores psum partition dim)
NKC = S // KC           # 16
SC = 512                # s chunk for projections / outproj
NSC = S // SC           # 4

_BUILT = {}


def _build_nc():
    nc = bacc.Bacc()

    xT_d = nc.declare_dram_parameter("xT", [D, S], BF16, isOutput=False)
    wqkv_d = nc.declare_dram_parameter("wqkv", [NPAIR, 128, 3, D], BF16, isOutput=False)
    wo_d = nc.declare_dram_parameter("wopk", [128, 8, NPAIR, 128], BF16, isOutput=False)
    ctab_d = nc.declare_dram_parameter("ctab", [128, S], BF16, isOutput=False)
    stab_d = nc.declare_dram_parameter("stab", [128, S], BF16, isOutput=False)
    mk_d = nc.declare_dram_parameter("masks", [KC, 2, 2 * QC], ADT, isOutput=False)
    idT_d = nc.declare_dram_parameter("identT", [128, 64], F32, isOutput=False)
    out_d = nc.declare_dram_parameter("outP", [D, S], F32, isOutput=True)

    swapmask = [i ^ 1 for i in range(32)]

    with TileContext(nc) as tc, ExitStack() as ctx:
        ep = ctx.enter_context
        consts = ep(tc.tile_pool(name="consts", bufs=1))
        xpool = ep(tc.tile_pool(name="xt", bufs=8))
        wpool = ep(tc.tile_pool(name="w", bufs=4))
        wopool = ep(tc.tile_pool(name="wo", bufs=1))
        vpool = ep(tc.tile_pool(name="vraw", bufs=1))
        rupool = ep(tc.tile_pool(name="ru", bufs=2))
        rvpool = ep(tc.tile_pool(name="rv", bufs=2))
        qkpool = ep(tc.tile_pool(name="qk", bufs=4))
        v1pool = ep(tc.tile_pool(name="v1", bufs=3))
        ppool = ep(tc.tile_pool(name="pT", bufs=6))
        orawpool = ep(tc.tile_pool(name="oraw", bufs=1))
        sumpool = ep(tc.tile_pool(name="sums", bufs=2))
        recpool = ep(tc.tile_pool(name="rec", bufs=2))
        otpool = ep(tc.tile_pool(name="oT", bufs=4))
        obpool = ep(tc.tile_pool(name="ob", bufs=4))
        drpool = ep(tc.tile_pool(name="dr", bufs=4, space="DRAM"))
        psA = ep(tc.tile_pool(name="psA", bufs=2, space="PSUM"))
        psB = ep(tc.tile_pool(name="psB", bufs=2, space="PSUM"))
        psO = ep(tc.tile_pool(name="psO", bufs=2, space="PSUM"))

        # ---- startup DMAs, ordered so the first proj matmuls start early ----
        xres = []
        xt = xpool.tile([128, S], BF16, tag="xt")
        nc.sync.dma_start(out=xt, in_=xT_d[0:128, :])
        xres.append(xt)

        wqkvs = []
        w0 = wpool.tile([128, 3, D], BF16, tag="w")
        nc.sync.dma_start(out=w0, in_=wqkv_d[0])
        wqkvs.append(w0)

        ctabs, stabs = [], []
        ct = consts.tile([128, SC], BF16, tag="ct0")
        st = consts.tile([128, SC], BF16, tag="st0")
        nc.sync.dma_start(out=ct, in_=ctab_d[:, 0:SC])
        nc.sync.dma_start(out=st, in_=stab_d[:, 0:SC])
        ctabs.append(ct)
        stabs.append(st)

        for ic in range(1, 8):
            xt = xpool.tile([128, S], BF16, tag="xt")
            nc.sync.dma_start(out=xt, in_=xT_d[ic * 128:(ic + 1) * 128, :])
            xres.append(xt)

        for sc in range(1, NSC):
            ct = consts.tile([128, SC], BF16, tag=f"ct{sc}")
            st = consts.tile([128, SC], BF16, tag=f"st{sc}")
            nc.sync.dma_start(out=ct, in_=ctab_d[:, sc * SC:(sc + 1) * SC])
            nc.sync.dma_start(out=st, in_=stab_d[:, sc * SC:(sc + 1) * SC])
            ctabs.append(ct)
            stabs.append(st)

        masks2 = consts.tile([KC, 2, 2 * QC], ADT)
        identT = consts.tile([128, 64], F32)
        nc.sync.dma_start(out=masks2, in_=mk_d[:, :, :])
        nc.sync.dma_start(out=identT, in_=idT_d[:, :])

        for hp in range(1, NPAIR):
            w = wpool.tile([128, 3, D], BF16, tag="w")
            nc.sync.dma_start(out=w, in_=wqkv_d[hp])
            wqkvs.append(w)
        wot = wopool.tile([128, 8, NPAIR, 128], BF16)
        nc.sync.dma_start(out=wot, in_=wo_d[:, :, :, :])

        oTs = []
        for hp in range(NPAIR):
            # ---------------- projections for head pair hp ----------------
            qT = qkpool.tile([128, S], ADT, tag="qk")
            kT = qkpool.tile([128, S], ADT, tag="qk")
            vraw = vpool.tile([128, S], F32, tag="vraw")

            _sc = nc.named_scope(f"proj{hp}"); _sc.__enter__()
            for sc in range(NSC):
                ssl = slice(sc * SC, (sc + 1) * SC)
                for pj in range(3):
                    ps = psA.tile([128, SC], F32, tag="psA")
                    for ic in range(8):
                        nc.tensor.matmul(
                            ps, wqkvs[hp][:, pj, ic * 128:(ic + 1) * 128],
                            xres[ic][:, ssl],
                            start=(ic == 0), stop=(ic == 7))
                    if pj < 2:  # Q or K: RoPE directly from PSUM
                        dst = qT if pj == 0 else kT
                        sh = rupool.tile([128, SC], F32, tag="ru")
                        nc.vector.stream_shuffle(out=sh, in_=ps, mask=swapmask)
                        t1 = rvpool.tile([128, SC], F32, tag="rv")
                        nc.vector.tensor_mul(out=t1, in0=ps, in1=ctabs[sc])
                        t2 = rupool.tile([128, SC], F32, tag="ru")
                        nc.vector.tensor_mul(out=t2, in0=sh, in1=stabs[sc])
                        nc.vector.tensor_add(out=dst[:, ssl], in0=t1, in1=t2)
                    else:       # V: drain to SBUF for PE transpose
                        nc.scalar.copy(out=vraw[:, ssl], in_=ps)

            _sc.__exit__(None, None, None)
            # ---------------- V transpose: [dk, s] -> [s, dk] + ones col ----
            _sc = nc.named_scope(f"vt{hp}"); _sc.__enter__()
            v1s = []
            for h in range(2):
                v1 = v1pool.tile([128, NKC, 65], ADT, tag="v1")
                for half in range(2):
                    pvt = psB.tile([128, 512], F32, tag="big")
                    for j in range(8):
                        kc = half * 8 + j
                        nc.tensor.transpose(
                            pvt[:, j * 64:(j + 1) * 64],
                            vraw[h * 64:(h + 1) * 64, kc * 128:(kc + 1) * 128],
                            identT[h * 64:(h + 1) * 64, 0:64])
                    nc.vector.tensor_copy(
                        out=v1[:, half * 8:(half + 1) * 8, 0:64],
                        in_=pvt.rearrange("p (kc d) -> p kc d", d=64))
                nc.vector.memset(v1[:, :, 64:65], 1.0)
                v1s.append(v1)

            _sc.__exit__(None, None, None)
            # ---------------- attention, both heads interleaved ----------------
            _sc = nc.named_scope(f"attn{hp}"); _sc.__enter__()
            oT = otpool.tile([128, S], ADT, tag="oT")
            oraw = orawpool.tile([128, S], F32, tag="oraw")
            hsl = (slice(0, 64), slice(64, 128))
            sums = [sumpool.tile([128, QC], F32, tag="sums", name=f"sums{hp}_{h}")
                    for h in range(2)]
            nc.vector.memset(sums[0], 1.0)
            nc.vector.memset(sums[1], 1.0)
            for qc in range(NQC):
                nact = 4 * qc + 4
                qsl = slice(qc * QC, (qc + 1) * QC)
                pquads = ([], [])
                for pr in range(nact // 2):
                    psqs = [psB.tile([128, 1024], F32, tag="big",
                                     name=f"psq{hp}_{qc}_{pr}_{h}")
                            for h in range(2)]
                    for sl in range(2):
                        kc = pr * 2 + sl
                        csl = slice(sl * QC, (sl + 1) * QC)
                        for h in range(2):
                            # two heads -> PE row groups (0,0) / (64,0), run
                            # concurrently (K=64 each)
                            nc.tensor.matmul(
                                psqs[h][:, csl],
                                kT[hsl[h], kc * KC:(kc + 1) * KC],
                                qT[hsl[h], qsl],
                                start=True, stop=True)
                    for h in range(2):
                        pq = ppool.tile([128, 1024], ADT, tag="pT")
                        nc.scalar.activation(
                            out=pq, in_=psqs[h],
                            func=mybir.ActivationFunctionType.Exp, scale=0.125)
                        if pr >= 2 * qc:  # diagonal pair: zero masked entries
                            nc.gpsimd.tensor_mul(
                                out=pq, in0=pq, in1=masks2[:, pr - 2 * qc, :])
                        pquads[h].append(pq)
                psos = [psO.tile([65, QC], F32, tag="psO",
                                 name=f"pso{hp}_{qc}_{h}") for h in range(2)]
                for kc in range(nact):
                    pr, sl = divmod(kc, 2)
                    csl = slice(sl * QC, (sl + 1) * QC)
                    for h in range(2):
                        nc.tensor.matmul(
                            psos[h], v1s[h][:, kc, :],
                            pquads[h][pr][:, csl],
                            start=(kc == 0), stop=(kc == nact - 1))
                for h in range(2):
                    nc.vector.tensor_copy(out=oraw[hsl[h], qsl],
                                          in_=psos[h][0:64, :])
                    nc.vector.tensor_copy(out=sums[h][32 * qc:32 * qc + 1, :],
                                          in_=psos[h][64:65, :])
            # batched normalization per head
            for h in range(2):
                hs = hsl[h]
                rec = recpool.tile([128, QC], F32, tag="rec")
                nc.vector.reciprocal(out=rec, in_=sums[h])
                drt = drpool.tile([NQC, QC], F32)
                for qc in range(NQC):
                    nc.sync.dma_start(out=drt[qc:qc + 1, :],
                                      in_=rec[32 * qc:32 * qc + 1, :])
                for qc in range(NQC):
                    qsl = slice(qc * QC, (qc + 1) * QC)
                    recB = recpool.tile([128, QC], F32, tag="recB")
                    nc.sync.dma_start(out=recB[hs, :],
                                      in_=drt[qc:qc + 1, :].to_broadcast((64, QC)))
                    nc.vector.tensor_mul(out=oT[hs, qsl], in0=oraw[hs, qsl],
                                         in1=recB[hs, :])
            _sc.__exit__(None, None, None)
            oTs.append(oT)

        # ---------------- output projection ----------------
        _sc = nc.named_scope("outproj"); _sc.__enter__()
        for oc in range(8):
            for sc in range(NSC):
                ps = psA.tile([128, SC], F32, tag="psA")
                for hp in range(NPAIR):
                    nc.tensor.matmul(ps, wot[:, oc, hp, :],
                                     oTs[hp][:, sc * SC:(sc + 1) * SC],
                                     start=(hp == 0), stop=(hp == NPAIR - 1))
                ob = obpool.tile([128, SC], F32, tag="ob")
                if sc % 2 == 0:
                    nc.vector.tensor_copy(out=ob, in_=ps)
                else:
                    nc.scalar.copy(out=ob, in_=ps)
                nc.scalar.dma_start(
                    out=out_d[oc * 128:(oc + 1) * 128, sc * SC:(sc + 1) * SC],
                    in_=ob)
        _sc.__exit__(None, None, None)

    nc.compile()
    return nc


def get_nc():
    if "nc" not in _BUILT:
        _BUILT["nc"] = _build_nc()
    return _BUILT["nc"]


def _host_prep(x, Wq, Wk, Wv, Wo, token_positions):
    bf = ml_dtypes.bfloat16
    pos = np.asarray(token_positions).astype(np.float32)
    half = DK // 2
    inv_freq = 1.0 / (10000.0 ** (np.arange(half, dtype=np.float32) * 2.0 / DK))
    ang = pos[:, None] * inv_freq[None, :]          # [S, 32]
    cos = np.cos(ang).astype(np.float32)            # [S, 32]
    sin = np.sin(ang).astype(np.float32)
    p = np.arange(128)
    j = (p % 64) // 2
    sign = np.where(p % 2 == 0, -1.0, 1.0).astype(np.float32)
    ctab = np.ascontiguousarray(cos[:, j].T).astype(bf)                 # [128, S]
    stab = np.ascontiguousarray(sin[:, j].T * sign[:, None]).astype(bf)

    kk = np.arange(KC)[:, None]
    qq = np.arange(QC)[None, :]
    # masks2[kk, i, sl*QC+qq] = keep(kk, qq, moff=2i+sl), multiplicative 0/1
    masks2 = np.stack(
        [np.concatenate([np.where(qq >= kk + 128 * (2 * i + sl), 1.0, 0.0)
                         for sl in range(2)], axis=1)
         for i in range(2)], axis=1).astype(bf)      # [KC, 2, 2*QC]
    identT = np.vstack([np.eye(64, dtype=np.float32)] * 2)

    in_maps = []
    for c in range(NCORES):
        b, hf = divmod(c, 2)
        m = {}
        m["xT"] = np.ascontiguousarray(x[b].T).astype(bf)  # [D, S]
        # wqkv[hp, p, pj, ic*128+j] = Wpj[hf*512 + hp*128 + j, ic*128 + p]
        wqkv = np.empty((NPAIR, 128, 3, D), dtype=bf)
        for pj, W in enumerate((Wq, Wk, Wv)):
            Ws = W[hf * 512:(hf + 1) * 512, :]       # [512 out, 1024 in]
            A = Ws.reshape(NPAIR, 128, 8, 128)       # [hp, jout, ic, pin]
            wqkv[:, :, pj, :] = A.transpose(0, 3, 2, 1).reshape(NPAIR, 128, D)
        m["wqkv"] = wqkv
        # wopk[p, oc, hp, j] = WoT[hp*128+p, oc*128+j];  WoT = Wo[:, cols].T
        WoT = Wo[:, hf * 512:(hf + 1) * 512].T       # [512, 1024]
        Bm = WoT.reshape(NPAIR, 128, 8, 128)         # [hp, p, oc, j]
        m["wopk"] = np.ascontiguousarray(
            Bm.transpose(1, 2, 0, 3)).astype(bf)     # [128, 8, NPAIR, 128]
        m["ctab"] = ctab
        m["stab"] = stab
        m["masks"] = masks2
        m["identT"] = identT
        in_maps.append(m)
    return in_maps


def run(inputs, trace=False, **kw):
    in_maps = _host_prep(**{k: np.asarray(v) for k, v in inputs.items()})
    nc = get_nc()
    res = run_bass_kernel_spmd(nc, in_maps, list(range(NCORES)), trace=trace, **kw)
    outs = [res.results[c]["outP"] for c in range(NCORES)]
    out = np.stack([(outs[2 * b] + outs[2 * b + 1]).T for b in range(B)])
    return out.astype(np.float32), res


def kernel(**inputs):
    out, _ = run(inputs, trace=False)
    return out


# revision 31
# speedup vs baseline: 1.1208x; 1.0883x over previous
"""Multi-head self-attention (B=4, S=2048, D=1024, H=16, RoPE, causal) on 8 trn2 cores.

Sharding: core c -> batch c//2, heads [8*(c%2), 8*(c%2)+8)   (2 cores per batch,
each doing 8 of the 16 heads).  Each core computes its partial output
projection out^T [1024, 2048]; host sums the two halves per batch and
transposes back.

All matmuls bf16 (f32 psum).  Layout is transposed throughout: x^T [D,S] in
SBUF (resident), Q^T/K^T [dk,s], scores^T [k,q] (softmax sum via a
ones-column appended to V in the attn@V matmul), out^T [o,s].

Scores matmuls have K=64 contraction: the two heads of a pair run
concurrently in PE row groups (0,0)/(64,0) via base-partition-derived
tile_position.  Causal masking: 0/1 bf16 multiply on P (gpsimd) instead of
-1e9 matmul adds.
"""
import sys
sys.path.insert(0, "/opt/trn_rl_repo")
import math
from contextlib import ExitStack
import numpy as np
import ml_dtypes

import concourse.bass as bass
import concourse.bacc as bacc
import concourse.mybir as mybir
from concourse.tile import TileContext
from concourse.bass_utils import run_bass_kernel_spmd

F32 = mybir.dt.float32
BF16 = mybir.dt.bfloat16
ADT = BF16

B, S, D, H, DK = 4, 2048, 1024, 16, 64
NCORES = 8
NPAIR = 4               # head pairs per core
QC = 512                # q chunk (matmul moving free size)
NQC = S // QC           # 4
KC = 128                # k chunk (scores psum partition dim)
NKC = S // KC           # 16
SC = 512                # s chunk for projections / outproj
NSC = S // SC           # 4

_BUILT = {}


def _build_nc():
    nc = bacc.Bacc()

    xT_d = nc.declare_dram_parameter("xT", [D, S], BF16, isOutput=False)
    wqkv_d = nc.declare_dram_parameter("wqkv", [NPAIR, 128, 3, D], BF16, isOutput=False)
    wo_d = nc.declare_dram_parameter("wopk", [128, 8, NPAIR, 128], BF16, isOutput=False)
    ctab_d = nc.declare_dram_parameter("ctab", [128, S], BF16, isOutput=False)
    stab_d = nc.declare_dram_parameter("stab", [128, S], BF16, isOutput=False)
    mk_d = nc.declare_dram_parameter("masks", [KC, KC], ADT, isOutput=False)
    idT_d = nc.declare_dram_parameter("identT", [128, 64], F32, isOutput=False)
    out_d = nc.declare_dram_parameter("outP", [D, S], F32, isOutput=True)

    swapmask = [i ^ 1 for i in range(32)]

    with TileContext(nc) as tc, ExitStack() as ctx:
        ep = ctx.enter_context
        consts = ep(tc.tile_pool(name="consts", bufs=1))
        xpool = ep(tc.tile_pool(name="xt", bufs=8))
        wpool = ep(tc.tile_pool(name="w", bufs=4))
        wopool = ep(tc.tile_pool(name="wo", bufs=1))
        vpool = ep(tc.tile_pool(name="vraw", bufs=1))
        rupool = ep(tc.tile_pool(name="ru", bufs=2))
        rvpool = ep(tc.tile_pool(name="rv", bufs=2))
        qkpool = ep(tc.tile_pool(name="qk", bufs=4))
        v1pool = ep(tc.tile_pool(name="v1", bufs=3))
        ppool = ep(tc.tile_pool(name="pT", bufs=6))
        orawpool = ep(tc.tile_pool(name="oraw", bufs=1))
        sumpool = ep(tc.tile_pool(name="sums", bufs=2))
        recpool = ep(tc.tile_pool(name="rec", bufs=2))
        otpool = ep(tc.tile_pool(name="oT", bufs=4))
        obpool = ep(tc.tile_pool(name="ob", bufs=4))
        drpool = ep(tc.tile_pool(name="dr", bufs=4, space="DRAM"))
        psA = ep(tc.tile_pool(name="psA", bufs=2, space="PSUM"))
        psB = ep(tc.tile_pool(name="psB", bufs=2, space="PSUM"))
        psO = ep(tc.tile_pool(name="psO", bufs=2, space="PSUM"))

        # ---- startup DMAs, ordered so the first proj matmuls start early ----
        xres = []
        xt = xpool.tile([128, S], BF16, tag="xt")
        nc.sync.dma_start(out=xt, in_=xT_d[0:128, :])
        xres.append(xt)

        wqkvs = []
        w0 = wpool.tile([128, 3, D], BF16, tag="w")
        nc.sync.dma_start(out=w0, in_=wqkv_d[0])
        wqkvs.append(w0)

        ctabs, stabs = [], []
        ct = consts.tile([128, SC], BF16, tag="ct0")
        st = consts.tile([128, SC], BF16, tag="st0")
        nc.sync.dma_start(out=ct, in_=ctab_d[:, 0:SC])
        nc.sync.dma_start(out=st, in_=stab_d[:, 0:SC])
        ctabs.append(ct)
        stabs.append(st)

        for ic in range(1, 8):
            xt = xpool.tile([128, S], BF16, tag="xt")
            nc.sync.dma_start(out=xt, in_=xT_d[ic * 128:(ic + 1) * 128, :])
            xres.append(xt)

        for sc in range(1, NSC):
            ct = consts.tile([128, SC], BF16, tag=f"ct{sc}")
            st = consts.tile([128, SC], BF16, tag=f"st{sc}")
            nc.sync.dma_start(out=ct, in_=ctab_d[:, sc * SC:(sc + 1) * SC])
            nc.sync.dma_start(out=st, in_=stab_d[:, sc * SC:(sc + 1) * SC])
            ctabs.append(ct)
            stabs.append(st)

        tri = consts.tile([KC, KC], ADT)
        identT = consts.tile([128, 64], F32)
        nc.sync.dma_start(out=tri, in_=mk_d[:, :])
        nc.sync.dma_start(out=identT, in_=idT_d[:, :])

        for hp in range(1, NPAIR):
            w = wpool.tile([128, 3, D], BF16, tag="w")
            nc.sync.dma_start(out=w, in_=wqkv_d[hp])
            wqkvs.append(w)
        wot = wopool.tile([128, 8, NPAIR, 128], BF16)
        nc.sync.dma_start(out=wot, in_=wo_d[:, :, :, :])

        oTs = []
        for hp in range(NPAIR):
            # ---------------- projections for head pair hp ----------------
            qT = qkpool.tile([128, S], ADT, tag="qk")
            kT = qkpool.tile([128, S], ADT, tag="qk")
            vraw = vpool.tile([128, S], F32, tag="vraw")

            _sc = nc.named_scope(f"proj{hp}"); _sc.__enter__()
            for sc in range(NSC):
                ssl = slice(sc * SC, (sc + 1) * SC)
                for pj in range(3):
                    ps = psA.tile([128, SC], F32, tag="psA")
                    for ic in range(8):
                        nc.tensor.matmul(
                            ps, wqkvs[hp][:, pj, ic * 128:(ic + 1) * 128],
                            xres[ic][:, ssl],
                            start=(ic == 0), stop=(ic == 7))
                    if pj < 2:  # Q or K: RoPE directly from PSUM
                        dst = qT if pj == 0 else kT
                        sh = rupool.tile([128, SC], F32, tag="ru")
                        nc.vector.stream_shuffle(out=sh, in_=ps, mask=swapmask)
                        t1 = rvpool.tile([128, SC], F32, tag="rv")
                        nc.vector.tensor_mul(out=t1, in0=ps, in1=ctabs[sc])
                        t2 = rupool.tile([128, SC], F32, tag="ru")
                        nc.vector.tensor_mul(out=t2, in0=sh, in1=stabs[sc])
                        nc.vector.tensor_add(out=dst[:, ssl], in0=t1, in1=t2)
                    else:       # V: drain to SBUF for PE transpose
                        nc.scalar.copy(out=vraw[:, ssl], in_=ps)

            _sc.__exit__(None, None, None)
            # ---------------- V transpose: [dk, s] -> [s, dk] + ones col ----
            _sc = nc.named_scope(f"vt{hp}"); _sc.__enter__()
            v1s = []
            for h in range(2):
                v1 = v1pool.tile([128, NKC, 65], ADT, tag="v1")
                for half in range(2):
                    pvt = psB.tile([128, 512], F32, tag="big")
                    for j in range(8):
                        kc = half * 8 + j
                        nc.tensor.transpose(
                            pvt[:, j * 64:(j + 1) * 64],
                            vraw[h * 64:(h + 1) * 64, kc * 128:(kc + 1) * 128],
                            identT[h * 64:(h + 1) * 64, 0:64])
                    nc.vector.tensor_copy(
                        out=v1[:, half * 8:(half + 1) * 8, 0:64],
                        in_=pvt.rearrange("p (kc d) -> p kc d", d=64))
                nc.vector.memset(v1[:, :, 64:65], 1.0)
                v1s.append(v1)

            _sc.__exit__(None, None, None)
            # ---------------- attention, both heads interleaved ----------------
            _sc = nc.named_scope(f"attn{hp}"); _sc.__enter__()
            oT = otpool.tile([128, S], ADT, tag="oT")
            oraw = orawpool.tile([128, S], F32, tag="oraw")
            hsl = (slice(0, 64), slice(64, 128))
            sums = [sumpool.tile([128, QC], F32, tag="sums", name=f"sums{hp}_{h}")
                    for h in range(2)]
            nc.vector.memset(sums[0], 1.0)
            nc.vector.memset(sums[1], 1.0)
            for qc in range(NQC):
                nact = 4 * qc + 4
                qsl = slice(qc * QC, (qc + 1) * QC)
                pquads = ([], [])
                for pr in range(nact // 2):
                    psqs = [psB.tile([128, 1024], F32, tag="big",
                                     name=f"psq{hp}_{qc}_{pr}_{h}")
                            for h in range(2)]
                    for sl in range(2):
                        kc = pr * 2 + sl
                        # diagonal block moff: cols [0, 128*moff) of this
                        # 512-q block are fully masked -> skip them in the
                        # matmul; cols [128m, 128m+128) get the tri multiply
                        m = kc - 4 * qc
                        lo = 128 * m if m > 0 else 0
                        csl = slice(sl * QC + lo, (sl + 1) * QC)
                        for h in range(2):
                            # two heads -> PE row groups (0,0) / (64,0), run
                            # concurrently (K=64 each)
                            nc.tensor.matmul(
                                psqs[h][:, csl],
                                kT[hsl[h], kc * KC:(kc + 1) * KC],
                                qT[hsl[h], qc * QC + lo:(qc + 1) * QC],
                                start=True, stop=True)
                    for h in range(2):
                        pq = ppool.tile([128, 1024], ADT, tag="pT")
                        nc.scalar.activation(
                            out=pq, in_=psqs[h],
                            func=mybir.ActivationFunctionType.Exp, scale=0.125)
                        for sl in range(2):
                            m = pr * 2 + sl - 4 * qc
                            if m >= 0:  # zero the staircase via tri multiply
                                a = sl * QC + 128 * m
                                nc.vector.tensor_mul(
                                    out=pq[:, a:a + 128],
                                    in0=pq[:, a:a + 128], in1=tri)
                        pquads[h].append(pq)
                psos = [psO.tile([65, QC], F32, tag="psO",
                                 name=f"pso{hp}_{qc}_{h}") for h in range(2)]
                for kc in range(nact):
                    pr, sl = divmod(kc, 2)
                    m = kc - 4 * qc
                    lo = 128 * m if m > 0 else 0
                    csl = slice(sl * QC + lo, (sl + 1) * QC)
                    for h in range(2):
                        nc.tensor.matmul(
                            psos[h][:, lo:QC], v1s[h][:, kc, :],
                            pquads[h][pr][:, csl],
                            start=(kc == 0), stop=(kc == nact - 1),
                            skip_group_check=True)
                for h in range(2):
                    nc.vector.tensor_copy(out=oraw[hsl[h], qsl],
                                          in_=psos[h][0:64, :])
                    nc.vector.tensor_copy(out=sums[h][32 * qc:32 * qc + 1, :],
                                          in_=psos[h][64:65, :])
            # batched normalization per head
            for h in range(2):
                hs = hsl[h]
                rec = recpool.tile([128, QC], F32, tag="rec")
                nc.vector.reciprocal(out=rec, in_=sums[h])
                drt = drpool.tile([NQC, QC], F32)
                for qc in range(NQC):
                    nc.sync.dma_start(out=drt[qc:qc + 1, :],
                                      in_=rec[32 * qc:32 * qc + 1, :])
                for qc in range(NQC):
                    qsl = slice(qc * QC, (qc + 1) * QC)
                    recB = recpool.tile([128, QC], F32, tag="recB")
                    nc.sync.dma_start(out=recB[hs, :],
                                      in_=drt[qc:qc + 1, :].to_broadcast((64, QC)))
                    nc.vector.tensor_mul(out=oT[hs, qsl], in0=oraw[hs, qsl],
                                         in1=recB[hs, :])
            _sc.__exit__(None, None, None)
            oTs.append(oT)

        # ---------------- output projection ----------------
        _sc = nc.named_scope("outproj"); _sc.__enter__()
        for oc in range(8):
            for sc in range(NSC):
                ps = psA.tile([128, SC], F32, tag="psA")
                for hp in range(NPAIR):
                    nc.tensor.matmul(ps, wot[:, oc, hp, :],
                                     oTs[hp][:, sc * SC:(sc + 1) * SC],
                                     start=(hp == 0), stop=(hp == NPAIR - 1))
                ob = obpool.tile([128, SC], F32, tag="ob")
                nc.vector.tensor_copy(out=ob, in_=ps)
                nc.sync.dma_start(
                    out=out_d[oc * 128:(oc + 1) * 128, sc * SC:(sc + 1) * SC],
                    in_=ob)
        _sc.__exit__(None, None, None)

    nc.compile()
    return nc


def get_nc():
    if "nc" not in _BUILT:
        _BUILT["nc"] = _build_nc()
    return _BUILT["nc"]


def _host_prep(x, Wq, Wk, Wv, Wo, token_positions):
    bf = ml_dtypes.bfloat16
    pos = np.asarray(token_positions).astype(np.float32)
    half = DK // 2
    inv_freq = 1.0 / (10000.0 ** (np.arange(half, dtype=np.float32) * 2.0 / DK))
    ang = pos[:, None] * inv_freq[None, :]          # [S, 32]
    cos = np.cos(ang).astype(np.float32)            # [S, 32]
    sin = np.sin(ang).astype(np.float32)
    p = np.arange(128)
    j = (p % 64) // 2
    sign = np.where(p % 2 == 0, -1.0, 1.0).astype(np.float32)
    ctab = np.ascontiguousarray(cos[:, j].T).astype(bf)                 # [128, S]
    stab = np.ascontiguousarray(sin[:, j].T * sign[:, None]).astype(bf)

    kk = np.arange(KC)[:, None]
    cc = np.arange(KC)[None, :]
    tri = np.where(cc >= kk, 1.0, 0.0).astype(bf)    # [KC, KC] keep c >= k
    identT = np.vstack([np.eye(64, dtype=np.float32)] * 2)

    in_maps = []
    for c in range(NCORES):
        b, hf = divmod(c, 2)
        m = {}
        m["xT"] = np.ascontiguousarray(x[b].T).astype(bf)  # [D, S]
        # wqkv[hp, p, pj, ic*128+j] = Wpj[hf*512 + hp*128 + j, ic*128 + p]
        wqkv = np.empty((NPAIR, 128, 3, D), dtype=bf)
        for pj, W in enumerate((Wq, Wk, Wv)):
            Ws = W[hf * 512:(hf + 1) * 512, :]       # [512 out, 1024 in]
            A = Ws.reshape(NPAIR, 128, 8, 128)       # [hp, jout, ic, pin]
            wqkv[:, :, pj, :] = A.transpose(0, 3, 2, 1).reshape(NPAIR, 128, D)
        m["wqkv"] = wqkv
        # wopk[p, oc, hp, j] = WoT[hp*128+p, oc*128+j];  WoT = Wo[:, cols].T
        WoT = Wo[:, hf * 512:(hf + 1) * 512].T       # [512, 1024]
        Bm = WoT.reshape(NPAIR, 128, 8, 128)         # [hp, p, oc, j]
        m["wopk"] = np.ascontiguousarray(
            Bm.transpose(1, 2, 0, 3)).astype(bf)     # [128, 8, NPAIR, 128]
        m["ctab"] = ctab
        m["stab"] = stab
        m["masks"] = tri
        m["identT"] = identT
        in_maps.append(m)
    return in_maps


def run(inputs, trace=False, **kw):
    in_maps = _host_prep(**{k: np.asarray(v) for k, v in inputs.items()})
    nc = get_nc()
    res = run_bass_kernel_spmd(nc, in_maps, list(range(NCORES)), trace=trace, **kw)
    outs = [res.results[c]["outP"] for c in range(NCORES)]
    out = np.stack([(outs[2 * b] + outs[2 * b + 1]).T for b in range(B)])
    return out.astype(np.float32), res


def kernel(**inputs):
    out, _ = run(inputs, trace=False)
    return out


# revision 34
# speedup vs baseline: 1.1785x; 1.0515x over previous
"""Multi-head self-attention (B=4, S=2048, D=1024, H=16, RoPE, causal) on 8 trn2 cores.

Sharding: core c -> batch c//2, heads [8*(c%2), 8*(c%2)+8)   (2 cores per batch,
each doing 8 of the 16 heads).  Each core computes its partial output
projection out^T [1024, 2048]; host sums the two halves per batch and
transposes back.

All matmuls bf16 (f32 psum).  Layout is transposed throughout: x^T [D,S] in
SBUF (resident), Q^T/K^T [dk,s], scores^T [k,q] (softmax sum via a
ones-column appended to V in the attn@V matmul), out^T [o,s].

Per kc, the two heads' scores land in one [128, h0|h1] psum tile: the two
K=64 matmuls run CONCURRENTLY in PE row groups (0,0)/(64,0), and one exp
covers both heads.  Causal masking: diagonal matmuls restrict their column
range (fully-masked part skipped) and a [128,128] triangular 0/1 multiply
zeroes the staircase.  proj/vt of head-pair hp+1 are software-pipelined into
the (scalar-bound) attention of hp; outproj is interleaved into attn3 per
q-chunk after per-chunk normalization.
"""
import sys
sys.path.insert(0, "/opt/trn_rl_repo")
import math
from contextlib import ExitStack
import numpy as np
import ml_dtypes

import concourse.bass as bass
import concourse.bacc as bacc
import concourse.mybir as mybir
from concourse.tile import TileContext
from concourse.bass_utils import run_bass_kernel_spmd

F32 = mybir.dt.float32
BF16 = mybir.dt.bfloat16
ADT = BF16

B, S, D, H, DK = 4, 2048, 1024, 16, 64
NCORES = 8
NPAIR = 4               # head pairs per core
QC = 512                # q chunk (matmul moving free size)
NQC = S // QC           # 4
KC = 128                # k chunk (scores psum partition dim)
NKC = S // KC           # 16
SC = 512                # s chunk for projections / outproj
NSC = S // SC           # 4

_BUILT = {}


def _build_nc():
    nc = bacc.Bacc()

    xT_d = nc.declare_dram_parameter("xT", [D, S], BF16, isOutput=False)
    wqkv_d = nc.declare_dram_parameter("wqkv", [NPAIR, 128, 3, D], BF16, isOutput=False)
    wo_d = nc.declare_dram_parameter("wopk", [128, 8, NPAIR, 128], BF16, isOutput=False)
    ctab_d = nc.declare_dram_parameter("ctab", [128, S], BF16, isOutput=False)
    stab_d = nc.declare_dram_parameter("stab", [128, S], BF16, isOutput=False)
    mk_d = nc.declare_dram_parameter("masks", [KC, KC], ADT, isOutput=False)
    idT_d = nc.declare_dram_parameter("identT", [128, 64], F32, isOutput=False)
    out_d = nc.declare_dram_parameter("outP", [D, S], F32, isOutput=True)

    swapmask = [i ^ 1 for i in range(32)]

    with TileContext(nc) as tc, ExitStack() as ctx:
        ep = ctx.enter_context
        consts = ep(tc.tile_pool(name="consts", bufs=1))
        xpool = ep(tc.tile_pool(name="xt", bufs=32))
        wpool = ep(tc.tile_pool(name="w", bufs=4))
        wopool = ep(tc.tile_pool(name="wo", bufs=1))
        vpool = ep(tc.tile_pool(name="vraw", bufs=1))
        rupool = ep(tc.tile_pool(name="ru", bufs=2))
        rvpool = ep(tc.tile_pool(name="rv", bufs=2))
        qkpool = ep(tc.tile_pool(name="qk", bufs=4))
        v1pool = ep(tc.tile_pool(name="v1", bufs=4))
        ppool = ep(tc.tile_pool(name="pT", bufs=6))
        orawpool = ep(tc.tile_pool(name="oraw", bufs=1))
        sumpool = ep(tc.tile_pool(name="sums", bufs=2))
        recpool = ep(tc.tile_pool(name="rec", bufs=2))
        otpool = ep(tc.tile_pool(name="oT", bufs=4))
        obpool = ep(tc.tile_pool(name="ob", bufs=4))
        drpool = ep(tc.tile_pool(name="dr", bufs=4, space="DRAM"))
        psA = ep(tc.tile_pool(name="psA", bufs=2, space="PSUM"))
        psB = ep(tc.tile_pool(name="psB", bufs=2, space="PSUM"))
        psO = ep(tc.tile_pool(name="psO", bufs=2, space="PSUM"))

        # ---- startup DMAs; x arrives per (ic, sc) chunk so the first
        # projection group is compute-ready at ~3us ----
        xres = [[None] * NSC for _ in range(8)]
        for ic in range(8):
            for sc in range(NSC):
                xt = xpool.tile([128, SC], BF16, tag="xt", name=f"x{ic}_{sc}")
                nc.sync.dma_start(
                    out=xt,
                    in_=xT_d[ic * 128:(ic + 1) * 128, sc * SC:(sc + 1) * SC])
                xres[ic][sc] = xt
            if ic == 0:
                wqkvs = []
                w0 = wpool.tile([128, 3, D], BF16, tag="w", name="wqkv0")
                nc.sync.dma_start(out=w0, in_=wqkv_d[0])
                wqkvs.append(w0)

        ctabs, stabs = [], []
        for sc in range(NSC):
            ct = consts.tile([128, SC], BF16, tag=f"ct{sc}", name=f"ct{sc}")
            st = consts.tile([128, SC], BF16, tag=f"st{sc}", name=f"st{sc}")
            nc.sync.dma_start(out=ct, in_=ctab_d[:, sc * SC:(sc + 1) * SC])
            nc.sync.dma_start(out=st, in_=stab_d[:, sc * SC:(sc + 1) * SC])
            ctabs.append(ct)
            stabs.append(st)

        tri = consts.tile([KC, KC], ADT)
        identT = consts.tile([128, 64], F32)
        nc.sync.dma_start(out=tri, in_=mk_d[:, :])
        nc.sync.dma_start(out=identT, in_=idT_d[:, :])

        for hp in range(1, NPAIR):
            w = wpool.tile([128, 3, D], BF16, tag="w", name=f"wqkv{hp}")
            nc.sync.dma_start(out=w, in_=wqkv_d[hp])
            wqkvs.append(w)
        wot = wopool.tile([128, 8, NPAIR, 128], BF16)
        nc.sync.dma_start(out=wot, in_=wo_d[:, :, :, :])

        state = {}   # hp -> (qT, kT, v1s)
        oTs = []

        def proj_steps(hp):
            """12 proj psum-group closures + 2 V-transpose closures."""
            qT = qkpool.tile([128, S], ADT, tag="qk", name=f"qT{hp}")
            kT = qkpool.tile([128, S], ADT, tag="qk", name=f"kT{hp}")
            vraw = vpool.tile([128, S], F32, tag="vraw", name=f"vraw{hp}")
            v1s = []
            state[hp] = (qT, kT, v1s)
            steps = []

            def pstep(sc, pj):
                with nc.named_scope(f"proj{hp}"):
                    ssl = slice(sc * SC, (sc + 1) * SC)
                    ps = psA.tile([128, SC], F32, tag="psA",
                                  name=f"pp{hp}_{sc}_{pj}")
                    for ic in range(8):
                        nc.tensor.matmul(
                            ps, wqkvs[hp][:, pj, ic * 128:(ic + 1) * 128],
                            xres[ic][sc],
                            start=(ic == 0), stop=(ic == 7))
                    if pj < 2:  # Q or K: RoPE directly from PSUM
                        dst = qT if pj == 0 else kT
                        sh = rupool.tile([128, SC], F32, tag="ru",
                                         name=f"sh{hp}_{sc}_{pj}")
                        nc.vector.stream_shuffle(out=sh, in_=ps, mask=swapmask)
                        t1 = rvpool.tile([128, SC], F32, tag="rv",
                                         name=f"t1{hp}_{sc}_{pj}")
                        nc.vector.tensor_mul(out=t1, in0=ps, in1=ctabs[sc])
                        t2 = rupool.tile([128, SC], F32, tag="ru",
                                         name=f"t2{hp}_{sc}_{pj}")
                        nc.vector.tensor_mul(out=t2, in0=sh, in1=stabs[sc])
                        nc.vector.tensor_add(out=dst[:, ssl], in0=t1, in1=t2)
                    else:       # V: drain to SBUF for PE transpose
                        nc.vector.tensor_copy(out=vraw[:, ssl], in_=ps)

            def vtstep(h):
                with nc.named_scope(f"vt{hp}"):
                    v1 = v1pool.tile([128, NKC, 65], ADT, tag="v1",
                                     name=f"v1_{hp}_{h}")
                    for half in range(2):
                        pvt = psB.tile([128, 512], F32, tag="big",
                                       name=f"pvt{hp}_{h}_{half}")
                        for j in range(8):
                            kc = half * 8 + j
                            nc.tensor.transpose(
                                pvt[:, j * 64:(j + 1) * 64],
                                vraw[h * 64:(h + 1) * 64, kc * 128:(kc + 1) * 128],
                                identT[h * 64:(h + 1) * 64, 0:64])
                        nc.vector.tensor_copy(
                            out=v1[:, half * 8:(half + 1) * 8, 0:64],
                            in_=pvt.rearrange("p (kc d) -> p kc d", d=64))
                    nc.vector.memset(v1[:, :, 64:65], 1.0)
                    v1s.append(v1)

            # V first so its transposes can start early; q/k order then by sc
            for sc in range(NSC):
                steps.append(lambda sc=sc: pstep(sc, 2))
            steps.append(lambda: vtstep(0))
            steps.append(lambda: vtstep(1))
            for sc in range(NSC):
                steps.append(lambda sc=sc: pstep(sc, 0))
                steps.append(lambda sc=sc: pstep(sc, 1))
            return steps

        def attention(hp, pending):
            """attn for hp; runs `pending` closures spread over the qc loop;
            for hp==3 interleaves the output projection per q-chunk."""
            qT, kT, v1s = state[hp]
            _sc = nc.named_scope(f"attn{hp}"); _sc.__enter__()
            oT = otpool.tile([128, S], ADT, tag="oT", name=f"oT{hp}")
            oraw = orawpool.tile([128, S], F32, tag="oraw", name=f"oraw{hp}")
            hsl = (slice(0, 64), slice(64, 128))
            sums = [sumpool.tile([128, QC], F32, tag="sums", name=f"sums{hp}_{h}")
                    for h in range(2)]
            recs = [recpool.tile([128, QC], F32, tag="rec", name=f"rec{hp}_{h}")
                    for h in range(2)]
            drts = [drpool.tile([NQC, QC], F32, name=f"drt{hp}_{h}")
                    for h in range(2)]
            nc.vector.memset(sums[0], 1.0)
            nc.vector.memset(sums[1], 1.0)
            nsteps = len(pending)
            for qc in range(NQC):
                nact = 4 * qc + 4
                qsl = slice(qc * QC, (qc + 1) * QC)
                pquads = []
                for kc in range(nact):
                    m = kc - 4 * qc
                    lo = 128 * m if m > 0 else 0
                    psq = psB.tile([128, 1024], F32, tag="big",
                                   name=f"psq{hp}_{qc}_{kc}")
                    for h in range(2):
                        nc.tensor.matmul(
                            psq[:, h * QC + lo:(h + 1) * QC],
                            kT[hsl[h], kc * KC:(kc + 1) * KC],
                            qT[hsl[h], qc * QC + lo:(qc + 1) * QC],
                            start=True, stop=True)
                    pq = ppool.tile([128, 1024], ADT, tag="pT",
                                    name=f"pq{hp}_{qc}_{kc}")
                    nc.scalar.activation(
                        out=pq, in_=psq,
                        func=mybir.ActivationFunctionType.Exp, scale=0.125)
                    if m >= 0:
                        for h in range(2):
                            a = h * QC + lo
                            nc.vector.tensor_mul(
                                out=pq[:, a:a + 128],
                                in0=pq[:, a:a + 128], in1=tri)
                    pquads.append(pq)
                psos = [psO.tile([65, QC], F32, tag="psO",
                                 name=f"pso{hp}_{qc}_{h}") for h in range(2)]
                for kc in range(nact):
                    m = kc - 4 * qc
                    lo = 128 * m if m > 0 else 0
                    for h in range(2):
                        nc.tensor.matmul(
                            psos[h][:, lo:QC], v1s[h][:, kc, :],
                            pquads[kc][:, h * QC + lo:(h + 1) * QC],
                            start=(kc == 0), stop=(kc == nact - 1),
                            skip_group_check=True)
                # normalization for this q-chunk (both heads)
                recB = recpool.tile([128, QC], F32, tag="recB",
                                    name=f"recB{hp}_{qc}")
                for h in range(2):
                    nc.vector.tensor_copy(out=oraw[hsl[h], qsl],
                                          in_=psos[h][0:64, :])
                    nc.vector.tensor_copy(out=sums[h][32 * qc:32 * qc + 1, :],
                                          in_=psos[h][64:65, :])
                    nc.vector.reciprocal(
                        out=recs[h][32 * qc:32 * qc + 1, :],
                        in_=sums[h][32 * qc:32 * qc + 1, :])
                    nc.sync.dma_start(out=drts[h][qc:qc + 1, :],
                                      in_=recs[h][32 * qc:32 * qc + 1, :])
                    nc.sync.dma_start(
                        out=recB[hsl[h], :],
                        in_=drts[h][qc:qc + 1, :].to_broadcast((64, QC)))
                nc.vector.tensor_mul(out=oT[:, qsl], in0=oraw[:, qsl],
                                     in1=recB)
                if hp == NPAIR - 1:
                    with nc.named_scope("outproj"):
                        for oc in range(8):
                            ps = psA.tile([128, SC], F32, tag="psA",
                                          name=f"ops{oc}_{qc}")
                            for h2 in range(NPAIR):
                                src = oTs[h2] if h2 < len(oTs) else oT
                                nc.tensor.matmul(
                                    ps, wot[:, oc, h2, :], src[:, qsl],
                                    start=(h2 == 0), stop=(h2 == NPAIR - 1))
                            ob = obpool.tile([128, SC], F32, tag="ob",
                                             name=f"ob{oc}_{qc}")
                            nc.vector.tensor_copy(out=ob, in_=ps)
                            nc.sync.dma_start(
                                out=out_d[oc * 128:(oc + 1) * 128, qsl],
                                in_=ob)
                else:
                    # run a slice of the next head-pair's proj/vt work
                    take = (nsteps * (qc + 1)) // NQC - (nsteps * qc) // NQC
                    for _ in range(take):
                        pending.pop(0)()
            _sc.__exit__(None, None, None)
            oTs.append(oT)

        # prologue: proj + vt for hp 0 runs undisturbed
        for st_ in proj_steps(0):
            st_()
        for hp in range(NPAIR):
            pending = proj_steps(hp + 1) if hp + 1 < NPAIR else []
            attention(hp, pending)

    nc.compile()
    return nc


def get_nc():
    if "nc" not in _BUILT:
        _BUILT["nc"] = _build_nc()
    return _BUILT["nc"]


def _host_prep(x, Wq, Wk, Wv, Wo, token_positions):
    bf = ml_dtypes.bfloat16
    pos = np.asarray(token_positions).astype(np.float32)
    half = DK // 2
    inv_freq = 1.0 / (10000.0 ** (np.arange(half, dtype=np.float32) * 2.0 / DK))
    ang = pos[:, None] * inv_freq[None, :]          # [S, 32]
    cos = np.cos(ang).astype(np.float32)            # [S, 32]
    sin = np.sin(ang).astype(np.float32)
    p = np.arange(128)
    j = (p % 64) // 2
    sign = np.where(p % 2 == 0, -1.0, 1.0).astype(np.float32)
    ctab = np.ascontiguousarray(cos[:, j].T).astype(bf)                 # [128, S]
    stab = np.ascontiguousarray(sin[:, j].T * sign[:, None]).astype(bf)

    kk = np.arange(KC)[:, None]
    cc = np.arange(KC)[None, :]
    tri = np.where(cc >= kk, 1.0, 0.0).astype(bf)    # [KC, KC] keep c >= k
    identT = np.vstack([np.eye(64, dtype=np.float32)] * 2)

    in_maps = []
    for c in range(NCORES):
        b, hf = divmod(c, 2)
        m = {}
        m["xT"] = np.ascontiguousarray(x[b].T).astype(bf)  # [D, S]
        # wqkv[hp, p, pj, ic*128+j] = Wpj[hf*512 + hp*128 + j, ic*128 + p]
        wqkv = np.empty((NPAIR, 128, 3, D), dtype=bf)
        for pj, W in enumerate((Wq, Wk, Wv)):
            Ws = W[hf * 512:(hf + 1) * 512, :]       # [512 out, 1024 in]
            A = Ws.reshape(NPAIR, 128, 8, 128)       # [hp, jout, ic, pin]
            wqkv[:, :, pj, :] = A.transpose(0, 3, 2, 1).reshape(NPAIR, 128, D)
        m["wqkv"] = wqkv
        # wopk[p, oc, hp, j] = WoT[hp*128+p, oc*128+j];  WoT = Wo[:, cols].T
        WoT = Wo[:, hf * 512:(hf + 1) * 512].T       # [512, 1024]
        Bm = WoT.reshape(NPAIR, 128, 8, 128)         # [hp, p, oc, j]
        m["wopk"] = np.ascontiguousarray(
            Bm.transpose(1, 2, 0, 3)).astype(bf)     # [128, 8, NPAIR, 128]
        m["ctab"] = ctab
        m["stab"] = stab
        m["masks"] = tri
        m["identT"] = identT
        in_maps.append(m)
    return in_maps


def run(inputs, trace=False, **kw):
    in_maps = _host_prep(**{k: np.asarray(v) for k, v in inputs.items()})
    nc = get_nc()
    res = run_bass_kernel_spmd(nc, in_maps, list(range(NCORES)), trace=trace, **kw)
    outs = [res.results[c]["outP"] for c in range(NCORES)]
    out = np.stack([(outs[2 * b] + outs[2 * b + 1]).T for b in range(B)])
    return out.astype(np.float32), res


def kernel(**inputs):
    out, _ = run(inputs, trace=False)
    return out
